# revision 28
# baseline (speedup 1.0000x reference)
"""Causal self-attention (B=4, T=2048, D=1024, 16 heads) on 8 TRN2 NeuronCores.

Sharding: tensor-parallel over heads — each core owns 2 heads (a 128-dim slice
of the QKV projections, column-parallel) and the matching 128 rows of W_O
(row-parallel). Each core computes a full-shape partial output; the host sums
the 8 partials.

Per-core dataflow (bf16 operands everywhere; fp32 PSUM accumulation):
  x.T [1024, 8192] bf16 (host-converted, streamed in 512-token chunks)
  Q.T/K.T = W.T-slice @ x.T  -> [128, 2048] per batch (d-major), computed
            k-tile-outer over chunk pairs so each weight load feeds 2 matmuls
  V'       = x-chunk-block (stationary) @ W_V.T-tile (moving) -> [128 tokens,
            128 dims] directly token-major: no PE transpose needed. Stored as
            a [128, 4*2*128] bf16 super-tile per chunk: per (key-block, head)
            a 128-col padded stationary slice (cols 0-63 V, col 64 ones for
            softmax sums, 65-127 junk -> junk PV rows never read).
  S.T     = K-block @ Q.T-chunk -> [128 keys, <=512 queries] per block, bf16
            at exact causal width
  P.T     = exp(S.T / 8) -> bf16 (ScalarE; no max-subtraction: scores ~
            N(0,1)); causal: above-diagonal blocks skipped, diagonal blocks
            masked by a constant upper-triangular 0/1 bf16 multiply (2x DVE)
  out.T   = V'.T @ P.T (bf16) accumulated over key blocks -> [128, 512] PSUM
            (row 64 = softmax sums, rows 65-127 junk)
  normalize: one reciprocal per chunk -> partition-broadcast per head ->
            multiply into bf16 ao
  out_partial.T = W_O-slice.T-chunk @ attnout -> [1024, 8192] bf16 streamed
            out (PSUM->SBUF evacuation alternates Vector/Scalar engines);
            host upcasts and sums the 8 partials

Projection matmuls for batch b+1 are interleaved between attention positions
of batch b to keep the tensor engine HAM-warm at 2.4 GHz.
"""
import os
import numpy as np
import ml_dtypes
import concourse.bacc as bacc
import concourse.mybir as mybir
import concourse.tile as tile
from concourse import bass_utils

B, T, D = 4, 2048, 1024
NH, DH = 16, 64
NC = 8
HPC = NH // NC        # 2 heads per core
CS = HPC * DH         # 128 projection dims per core
TOK = B * T           # 8192 tokens
QC = 512              # query-chunk width
NCH = T // QC         # 4 chunks per batch
KT = D // 128         # 8 contraction tiles
NKB = T // 128        # 16 key blocks per batch
f32 = mybir.dt.float32
bf16 = mybir.dt.bfloat16
AFT = mybir.ActivationFunctionType
SCALE = float(1.0 / np.sqrt(DH))

_cache = {}


def _build():
    if "nc" in _cache:
        return _cache["nc"]
    nc = bacc.Bacc("TRN2", target_bir_lowering=False, debug=False)

    xT_d = nc.dram_tensor("xT", [D, TOK], bf16, kind="ExternalInput").ap()
    WQT_d = nc.dram_tensor("WQT", [D, CS], bf16, kind="ExternalInput").ap()
    WKT_d = nc.dram_tensor("WKT", [D, CS], bf16, kind="ExternalInput").ap()
    WVT_d = nc.dram_tensor("WVT", [D, CS], bf16, kind="ExternalInput").ap()
    WOT_d = nc.dram_tensor("WOT", [CS, D], bf16, kind="ExternalInput").ap()
    umask_d = nc.dram_tensor("umask", [128, 128], bf16, kind="ExternalInput").ap()
    ones_d = nc.dram_tensor("onesc", [128, 8], bf16, kind="ExternalInput").ap()
    out_d = nc.dram_tensor("outT", [D, TOK], bf16, kind="ExternalOutput").ap()

    with tile.TileContext(nc) as tc:
      with nc.allow_low_precision(reason="bf16 attention"):
        with tc.tile_pool(name="sb", bufs=1) as sb, \
             tc.tile_pool(name="sp", bufs=2) as sp, \
             tc.tile_pool(name="ps", bufs=1, space="PSUM") as ps:
            # ---- constants / weights (persistent)
            WQT_t = sb.tile([128, KT * CS], bf16, tag="wqt")
            WKT_t = sb.tile([128, KT * CS], bf16, tag="wkt")
            WVT_t = sb.tile([128, KT * CS], bf16, tag="wvt")
            # one trigger per weight tensor (3D access pattern folds the
            # k-tiles); off the sync queue: sync carries the latency-critical
            # first x-chunk loads at startup
            # startup choreography: each queue's first trigger is what the first
            # matmuls need (WQT k0 on gpsimd, x k0 on sync). The first Q matmul
            # needs only WQT k0/k1, so WQT stays per-k-pair granular.
            for k0 in range(0, KT, 2):
                nc.gpsimd.dma_start(out=WQT_t[:, k0 * CS:(k0 + 2) * CS].rearrange("p (k c) -> p k c", k=2),
                                    in_=WQT_d[k0 * 128:(k0 + 2) * 128, :].rearrange("(k p) c -> p k c", k=2))
            for k0 in range(0, KT, 4):
                nc.scalar.dma_start(out=WKT_t[:, k0 * CS:(k0 + 4) * CS].rearrange("p (k c) -> p k c", k=4),
                                    in_=WKT_d[k0 * 128:(k0 + 4) * 128, :].rearrange("(k p) c -> p k c", k=4))
            nc.gpsimd.dma_start(out=WVT_t[:].rearrange("p (k c) -> p k c", k=KT),
                                in_=WVT_d[:].rearrange("(k p) c -> p k c", k=KT))
            WOT_t = sb.tile([128, D], bf16, tag="wot")
            nc.gpsimd.dma_start(out=WOT_t[:], in_=WOT_d[:, :])
            umask_t = sb.tile([128, 128], bf16, tag="umask")
            nc.scalar.dma_start(out=umask_t[:], in_=umask_d[:, :])
            ones_t = sb.tile([128, 8], bf16, tag="ones")
            nc.scalar.dma_start(out=ones_t[:], in_=ones_d[:, :])
            # warm the ScalarE exp table set during the startup DMA wait so the
            # first real exp doesn't pay the ~2.7us ACT_TABLE_LOAD
            warm = sp.tile([1, 2], f32, tag="warm", bufs=1)
            nc.scalar.activation(warm[:], ones_t[0:1, 0:2], AFT.Exp, scale=1.0)

            qz = {}   # per-batch zero-padded Q.T pair: qz[b][h] [128, T] bf16
                      # (head h's 64 rows live, other 64 rows zero, so scores
                      # run as full K=128 matmuls sharing one K stationary)
            kt = {}   # per-batch K.T [128, T]
            vp = {}   # (b, ch) -> V' super-tile [128, 4*2*128] bf16

            def proj_steps(b, chp):
                """QKV projection for chunk pair chp (chunks 2chp, 2chp+1) of
                batch b as emit-closures, interleaved between attention
                positions. Q/K run k-tile-outer so one weight load feeds both
                chunks' matmuls."""
                chunks = (2 * chp, 2 * chp + 1)
                gs = [NCH * b + ch for ch in chunks]
                if chp == 0:
                    qz[b] = [sp.tile([128, T], bf16, tag=f"qz{h}", name=f"qz{h}_{b}", bufs=2)
                             for h in range(HPC)]
                    kt[b] = sp.tile([128, T], bf16, tag="kt", name=f"kt{b}", bufs=2)
                    # zero the dead halves (read by the K=128 scores matmuls).
                    # Only the first two batches: the tag rotates over 2 slots
                    # and the zero halves are never overwritten, so later
                    # batches inherit them.
                    if b < 2:
                        nc.vector.memset(qz[b][0][DH:128, :], 0.0)
                        nc.vector.memset(qz[b][1][0:DH, :], 0.0)
                # all 8 k-tiles of the chunk pair in one super-tile
                xt_s = sp.tile([128, KT * 2 * QC], bf16, tag="xt", name=f"xt{b}_{chp}", bufs=2)
                xts = [xt_s[:, k * 2 * QC:(k + 1) * 2 * QC] for k in range(KT)]
                src = xT_d[:].rearrange("(k p) t -> p k t", k=KT)[:, :, gs[0] * QC:(gs[0] + 2) * QC]
                x3 = xt_s[:].rearrange("p (k t) -> p k t", k=KT)

                def load_x():
                    if b == 0:
                        # startup: spread the k-tiles over parallel queues so
                        # the first projection matmuls start ASAP (pair 1 too:
                        # it's consumed within ~30us, before a 2-trigger load
                        # could finish). The first k-tiles of pair 0 go in
                        # half-chunk pieces so the very first matmul's input
                        # lands in ~1us of transfer.
                        if chp == 0:
                            for k in range(4):
                                eng = (nc.sync, nc.scalar, nc.gpsimd)[k % 3]
                                eng.dma_start(out=x3[:, k, 0:QC], in_=src[:, k, 0:QC])
                                eng.dma_start(out=x3[:, k, QC:2 * QC], in_=src[:, k, QC:2 * QC])
                            for k in range(4, KT):
                                eng = (nc.sync, nc.scalar, nc.gpsimd)[k % 3]
                                eng.dma_start(out=x3[:, k, :], in_=src[:, k, :])
                        else:
                            for k in range(KT):
                                eng = (nc.sync, nc.scalar, nc.gpsimd)[k % 3]
                                eng.dma_start(out=x3[:, k, :], in_=src[:, k, :])
                    else:
                        nc.sync.dma_start(out=x3[:, 0:KT // 2, :], in_=src[:, 0:KT // 2, :])
                        nc.gpsimd.dma_start(out=x3[:, KT // 2:KT, :], in_=src[:, KT // 2:KT, :])
                steps = [load_x]

                # Q/K: k-outer over the chunk pair; two PSUM accumulators
                def evac_q(pps, ci, ch):
                    for h in range(HPC):
                        nc.vector.tensor_copy(qz[b][h][h * DH:(h + 1) * DH, ch * QC:(ch + 1) * QC],
                                              pps[ci][h * DH:(h + 1) * DH, :])

                def evac_k(pps, ci, ch):
                    nc.vector.tensor_copy(kt[b][:, ch * QC:(ch + 1) * QC], pps[ci][:])

                for wt, evac, nm in ((WQT_t, evac_q, "q"), (WKT_t, evac_k, "k")):
                    pps = [ps.tile([128, QC], f32, tag="mm", name=f"pp{nm}{g}", bufs=2) for g in gs]
                    for k0 in range(0, KT, 2):
                        def fqk(wt=wt, evac=evac, k0=k0, pps=pps):
                            for k in (k0, k0 + 1):
                                for ci in range(2):
                                    nc.tensor.matmul(pps[ci][:], wt[:, k * CS:(k + 1) * CS],
                                                     xts[k][:, ci * QC:(ci + 1) * QC],
                                                     start=(k == 0), stop=(k == KT - 1))
                            if k0 + 2 == KT:
                                for ci, ch in enumerate(chunks):
                                    evac(pps, ci, ch)
                        steps.append(fqk)

                # V' directly token-major: x-block stationary, W_V.T moving.
                # out [128 tokens, 128 dims] per token-block, 4 blocks per
                # chunk packed into one [128, 512] PSUM tile as column stripes.
                for ci, ch in enumerate(chunks):
                    g = gs[ci]
                    vps = ps.tile([128, QC], f32, tag="mm", name=f"vps{g}", bufs=2)
                    vpt = sp.tile([128, 4 * 2 * 128], bf16, tag="vp", name=f"vp{b}_{ch}", bufs=2 * NCH)
                    vp[(b, ch)] = vpt
                    for tt0 in range(0, 4, 2):
                        def fv(tt0=tt0, vps=vps, vpt=vpt, ci=ci, g=g):
                            for tt in (tt0, tt0 + 1):
                                for k in range(KT):
                                    nc.tensor.matmul(vps[:, tt * 128:(tt + 1) * 128],
                                                     xts[k][:, ci * QC + tt * 128:ci * QC + (tt + 1) * 128],
                                                     WVT_t[:, k * CS:(k + 1) * CS],
                                                     start=(k == 0), stop=(k == KT - 1))
                            if tt0 == 2:
                                vp4s = vps[:].rearrange("p (t h x) -> p t h x", t=4, h=2)
                                vp4 = vpt[:].rearrange("p (t h x) -> p t h x", t=4, h=2)
                                nc.vector.tensor_copy(vp4[:, :, :, 0:DH], vp4s[:, :, :, 0:DH])
                                nc.gpsimd.tensor_copy(
                                    vp4[:, :, :, DH:DH + 1],
                                    ones_t[:, 0:8].rearrange("p (t h x) -> p t h x", t=4, h=2))
                        steps.append(fv)
                return steps

            pending = []  # queued proj closures, interleaved into attention

            def pull(n):
                for _ in range(min(n, len(pending))):
                    pending.pop(0)()

            def oproj_steps(g, ao, final=False):
                """O-projection for chunk g as filler steps (2 output tiles each).
                PSUM evacuation alternates Vector/Scalar to split the load.
                Output DMA is consolidated to 2 triggers per chunk (one per
                half), except the final chunk which streams per-tile so the
                kernel tail stays short."""
                ot = sp.tile([128, 8 * QC], bf16, tag="ot", name=f"ot{g}", bufs=2)
                dst = out_d[:].rearrange("(m p) t -> p m t", m=8)[:, :, g * QC:(g + 1) * QC]
                steps = []
                for mt in range(8):
                    def fo(mt=mt):
                        op = ps.tile([128, QC], f32, tag="mm", name=f"op{g}_{mt}", bufs=2)
                        nc.tensor.matmul(op[:], WOT_t[:, mt * 128:(mt + 1) * 128], ao[:],
                                         start=True, stop=True)
                        osl = ot[:, mt * QC:(mt + 1) * QC]
                        if mt % 2 == 1:
                            nc.scalar.copy(osl, op[:])
                        else:
                            nc.vector.tensor_copy(osl, op[:])
                        if final:
                            nc.sync.dma_start(out=dst[:, mt, :], in_=osl)
                        elif mt == 3:
                            nc.sync.dma_start(
                                out=dst[:, 0:4, :],
                                in_=ot[:, 0:4 * QC].rearrange("p (m t) -> p m t", m=4))
                        elif mt == 7:
                            nc.gpsimd.dma_start(
                                out=dst[:, 4:8, :],
                                in_=ot[:, 4 * QC:].rearrange("p (m t) -> p m t", m=4))
                    steps.append(fo)
                return steps

            def attn_chunk(b, ch, oproj_prev, pos_left=40):
                """Attention + normalize for query chunk ch of batch b.
                Two-stage software pipeline: scores/exp for kb+1 are issued before
                the PV matmuls of kb, so the PV weight-loads never wait on exp.
                oproj_prev = (g, ao) of the previous chunk, interleaved here."""
                g = NCH * b + ch
                pvs = [ps.tile([128, QC], f32, tag=f"pv{h}", name=f"pv{h}_{g}", bufs=1) for h in range(HPC)]
                nkb = 4 * ch + 4

                def scores(kb):
                    off = max(0, 128 * kb - QC * ch)
                    sc = ps.tile([128, 2 * QC], f32, tag="sc", name=f"sc{g}_{kb}", bufs=2)
                    pt = sp.tile([128, 2 * QC], bf16, tag="pt", name=f"pt{g}_{kb}", bufs=6)
                    for h in range(HPC):
                        hb = h * QC
                        # full K=128 contraction: head h's dead 64 rows of qz are
                        # zero, so the shared K stationary contributes nothing
                        # from the other head's rows
                        nc.tensor.matmul(sc[:, hb + off:hb + QC],
                                         kt[b][:, kb * 128:(kb + 1) * 128],
                                         qz[b][h][:, ch * QC + off:(ch + 1) * QC],
                                         start=True, stop=True)
                    if off == 0:
                        nc.scalar.activation(pt[:], sc[:], AFT.Exp, scale=SCALE)
                    else:
                        sc3 = sc[:].rearrange("p (h x) -> p h x", h=2)[:, :, off:QC]
                        pt3e = pt[:].rearrange("p (h x) -> p h x", h=2)[:, :, off:QC]
                        nc.scalar.activation(pt3e, sc3, AFT.Exp, scale=SCALE)
                    if 128 * kb >= QC * ch:  # diagonal block: mask keys > queries
                        pt3 = pt[:].rearrange("p (h x) -> p h x", h=2)[:, :, off:off + 128]
                        nc.vector.tensor_mul(pt3, pt3, umask_t[:].rearrange("p (o x) -> p o x", o=1).broadcast_to([128, 2, 128]))
                    return pt, off

                def pv_mm(kb, pt, off):
                    vch, vtt = kb // 4, kb % 4
                    vpt = vp[(b, vch)][:].rearrange("p (t h x) -> p t h x", t=4, h=2)
                    for h in range(HPC):
                        hb = h * QC
                        nc.tensor.matmul(pvs[h][:, off:QC],
                                         vpt[:, vtt, h, :],
                                         pt[:, hb + off:hb + QC],
                                         start=(kb == 0), stop=(kb == nkb - 1))

                if oproj_prev is not None:
                    # back of the queue: O-proj has no same-chunk consumers, so it
                    # backlogs into the last batch's attention (which has no next
                    # projection to interleave and is otherwise exp-bound)
                    pending.extend(oproj_steps(*oproj_prev))
                q0 = scores(0)
                q1 = scores(1)
                for kb in range(2, nkb):
                    cur = scores(kb)
                    # pace the projection/O-proj filler so the PE has dense
                    # work through the exp-bound deep chunks of the batch
                    pull(2 if len(pending) >= pos_left - (kb - 2) else 1)
                    pv_mm(kb - 2, *q0)
                    q0, q1 = q1, cur
                pull(1)
                pv_mm(nkb - 2, *q0)
                pull(1)
                pv_mm(nkb - 1, *q1)
                # normalize -> attnout [128, 512] bf16; one reciprocal per chunk
                ao = sp.tile([128, QC], bf16, tag="ao", name=f"ao{g}", bufs=8)
                s_g = sp.tile([1, 2 * QC], f32, tag="sh", name=f"sh{g}", bufs=3)
                r_g = sp.tile([1, 2 * QC], f32, tag="rh", name=f"rh{g}", bufs=3)
                for h in range(HPC):
                    # sums-row copy on ScalarE: keeps DVE free and lets the
                    # reciprocal start sooner
                    nc.scalar.copy(s_g[0:1, h * QC:(h + 1) * QC], pvs[h][64:65, :])
                nc.vector.reciprocal_approx_fast(out=r_g[0:1, :], in_=s_g[0:1, :])
                for h in range(HPC):
                    bc = sp.tile([DH, QC], f32, tag="bc", name=f"bc{g}_{h}", bufs=3)
                    nc.gpsimd.partition_broadcast(bc[:], r_g[0:1, h * QC:(h + 1) * QC])
                    nc.vector.tensor_mul(ao[h * DH:(h + 1) * DH, :], pvs[h][0:DH, :], bc[:])
                return (g, ao)

            # emission: proj(0) pair 0 eagerly, then attention starts right away
            # (chunk 0 only needs pair-0 results); pair 1 interleaves into the
            # early attention chunks via the pending queue. Both pairs' x loads
            # are issued first so pair 1's DMA overlaps pair 0's matmuls.
            p0 = proj_steps(0, 0)
            p1 = proj_steps(0, 1)
            p0[0]()
            p1[0]()
            for s in p0[1:]:
                s()
            pending.extend(p1[1:])
            oprev = None
            for b in range(B):
                if b + 1 < B:
                    for chp in range(NCH // 2):
                        pending.extend(proj_steps(b + 1, chp))
                for ch in range(NCH):
                    pos_left = sum(4 * c + 4 for c in range(ch, NCH))
                    oprev = attn_chunk(b, ch, oprev, pos_left)
                # drain most pending work, but hold tail steps back as PE filler
                # across the batch boundary (late-chunk proj of b+1 and O-proj
                # backlog, none needed by b+1's first attention chunk)
                pull(max(0, len(pending) - 16))
            pull(len(pending))  # leftover O-proj backlog
            for s in oproj_steps(*oprev, final=True):
                s()

    nc.compile()
    _cache["nc"] = nc
    return nc


def kernel(x, W_Q, W_K, W_V, W_O):
    nc = _build()
    bf = ml_dtypes.bfloat16
    xT = np.ascontiguousarray(np.asarray(x, dtype=np.float32).reshape(TOK, D).T).astype(bf)
    umask = np.triu(np.ones((128, 128), dtype=np.float32)).astype(bf)
    in_maps = []
    for c in range(NC):
        cs = slice(c * CS, (c + 1) * CS)
        in_maps.append({
            "xT": xT,
            "WQT": np.ascontiguousarray(np.asarray(W_Q, dtype=np.float32)[cs].T).astype(bf),
            "WKT": np.ascontiguousarray(np.asarray(W_K, dtype=np.float32)[cs].T).astype(bf),
            "WVT": np.ascontiguousarray(np.asarray(W_V, dtype=np.float32)[cs].T).astype(bf),
            "WOT": np.ascontiguousarray(np.asarray(W_O, dtype=np.float32)[:, cs].T).astype(bf),
            "umask": umask,
            "onesc": np.ones((128, 8), dtype=np.float32).astype(bf),
        })
    trace = bool(os.environ.get("KERNEL_TRACE"))
    res = bass_utils.run_bass_kernel_spmd(nc, in_maps, list(range(NC)), trace=trace)
    kernel.last_result = res
    out = np.zeros((D, TOK), dtype=np.float64)
    for c in range(NC):
        out += res.results[c]["outT"].astype(np.float64)
    return np.ascontiguousarray(out.T.reshape(B, T, D)).astype(np.float32)


# revision 29
# speedup vs baseline: 1.0398x; 1.0398x over previous
"""Causal self-attention (B=4, T=2048, D=1024, 16 heads) on 8 TRN2 NeuronCores.

Sharding: tensor-parallel over heads — each core owns 2 heads (a 128-dim slice
of the QKV projections, column-parallel) and the matching 128 rows of W_O
(row-parallel). Each core computes a full-shape partial output; the host sums
the 8 partials.

Per-core dataflow (bf16 operands everywhere; fp32 PSUM accumulation):
  x.T [1024, 8192] bf16 (host-converted, streamed in 512-token chunks)
  Q.T/K.T = W.T-slice @ x.T  -> [128, 2048] per batch (d-major), computed
            k-tile-outer over chunk pairs so each weight load feeds 2 matmuls
  V'       = x-chunk-block (stationary) @ W_V.T-tile (moving) -> [128 tokens,
            128 dims] directly token-major: no PE transpose needed. Stored as
            a [128, 4*2*128] bf16 super-tile per chunk: per (key-block, head)
            a 128-col padded stationary slice (cols 0-63 V, col 64 ones for
            softmax sums, 65-127 junk -> junk PV rows never read).
  S.T     = K-block @ Q.T-chunk -> [128 keys, <=512 queries] per block, bf16
            at exact causal width
  P.T     = exp(S.T / 8) -> bf16 (ScalarE; no max-subtraction: scores ~
            N(0,1)); causal: above-diagonal blocks skipped, diagonal blocks
            masked by a constant upper-triangular 0/1 bf16 multiply (2x DVE)
  out.T   = V'.T @ P.T (bf16) accumulated over key blocks -> [128, 512] PSUM
            (row 64 = softmax sums, rows 65-127 junk)
  normalize: one reciprocal per chunk -> partition-broadcast per head ->
            multiply into bf16 ao
  out_partial.T = W_O-slice.T-chunk @ attnout -> [1024, 8192] bf16 streamed
            out (PSUM->SBUF evacuation alternates Vector/Scalar engines);
            host upcasts and sums the 8 partials

Projection matmuls for batch b+1 are interleaved between attention positions
of batch b to keep the tensor engine HAM-warm at 2.4 GHz.
"""
import os
import numpy as np
import ml_dtypes
import concourse.bacc as bacc
import concourse.mybir as mybir
import concourse.tile as tile
from concourse import bass_utils

B, T, D = 4, 2048, 1024
NH, DH = 16, 64
NC = 8
HPC = NH // NC        # 2 heads per core
CS = HPC * DH         # 128 projection dims per core
TOK = B * T           # 8192 tokens
QC = 512              # query-chunk width
NCH = T // QC         # 4 chunks per batch
KT = D // 128         # 8 contraction tiles
NKB = T // 128        # 16 key blocks per batch
f32 = mybir.dt.float32
bf16 = mybir.dt.bfloat16
AFT = mybir.ActivationFunctionType
SCALE = float(1.0 / np.sqrt(DH))

_cache = {}


def _build():
    if "nc" in _cache:
        return _cache["nc"]
    nc = bacc.Bacc("TRN2", target_bir_lowering=False, debug=False)

    xT_d = nc.dram_tensor("xT", [D, TOK], bf16, kind="ExternalInput").ap()
    WQT_d = nc.dram_tensor("WQT", [D, CS], bf16, kind="ExternalInput").ap()
    WKT_d = nc.dram_tensor("WKT", [D, CS], bf16, kind="ExternalInput").ap()
    WVT_d = nc.dram_tensor("WVT", [D, CS], bf16, kind="ExternalInput").ap()
    WOT_d = nc.dram_tensor("WOT", [CS, D], bf16, kind="ExternalInput").ap()
    umask_d = nc.dram_tensor("umask", [128, 128], bf16, kind="ExternalInput").ap()
    ones_d = nc.dram_tensor("onesc", [128, 8], bf16, kind="ExternalInput").ap()
    out_d = nc.dram_tensor("outT", [D, TOK], bf16, kind="ExternalOutput").ap()

    with tile.TileContext(nc) as tc:
      with nc.allow_low_precision(reason="bf16 attention"):
        with tc.tile_pool(name="sb", bufs=1) as sb, \
             tc.tile_pool(name="sp", bufs=2) as sp, \
             tc.tile_pool(name="ps", bufs=1, space="PSUM") as ps:
            # ---- constants / weights (persistent)
            WQT_t = sb.tile([128, KT * CS], bf16, tag="wqt")
            WKT_t = sb.tile([128, KT * CS], bf16, tag="wkt")
            WVT_t = sb.tile([128, KT * CS], bf16, tag="wvt")
            # one trigger per weight tensor (3D access pattern folds the
            # k-tiles); off the sync queue: sync carries the latency-critical
            # first x-chunk loads at startup
            # startup choreography: each queue's first trigger is what the first
            # matmuls need (WQT k0 on gpsimd, x k0 on sync). The first Q matmul
            # needs only WQT k0/k1, so WQT stays per-k-pair granular.
            for k0 in range(0, KT, 2):
                nc.gpsimd.dma_start(out=WQT_t[:, k0 * CS:(k0 + 2) * CS].rearrange("p (k c) -> p k c", k=2),
                                    in_=WQT_d[k0 * 128:(k0 + 2) * 128, :].rearrange("(k p) c -> p k c", k=2))
            for k0 in range(0, KT, 4):
                nc.scalar.dma_start(out=WKT_t[:, k0 * CS:(k0 + 4) * CS].rearrange("p (k c) -> p k c", k=4),
                                    in_=WKT_d[k0 * 128:(k0 + 4) * 128, :].rearrange("(k p) c -> p k c", k=4))
            nc.gpsimd.dma_start(out=WVT_t[:].rearrange("p (k c) -> p k c", k=KT),
                                in_=WVT_d[:].rearrange("(k p) c -> p k c", k=KT))
            WOT_t = sb.tile([128, D], bf16, tag="wot")
            nc.gpsimd.dma_start(out=WOT_t[:], in_=WOT_d[:, :])
            umask_t = sb.tile([128, 128], bf16, tag="umask")
            nc.scalar.dma_start(out=umask_t[:], in_=umask_d[:, :])
            ones_t = sb.tile([128, 8], bf16, tag="ones")
            nc.scalar.dma_start(out=ones_t[:], in_=ones_d[:, :])
            # warm the ScalarE exp table set during the startup DMA wait so the
            # first real exp doesn't pay the ~2.7us ACT_TABLE_LOAD
            warm = sp.tile([1, 2], f32, tag="warm", bufs=1)
            nc.scalar.activation(warm[:], ones_t[0:1, 0:2], AFT.Exp, scale=1.0)

            qz = {}   # per-batch zero-padded Q.T pair: qz[b][h] [128, T] bf16
                      # (head h's 64 rows live, other 64 rows zero, so scores
                      # run as full K=128 matmuls sharing one K stationary)
            kt = {}   # per-batch K.T [128, T]
            vp = {}   # (b, ch) -> V' super-tile [128, 4*2*128] bf16

            def proj_steps(b, chp):
                """QKV projection for chunk pair chp (chunks 2chp, 2chp+1) of
                batch b as emit-closures, interleaved between attention
                positions. Q/K run k-tile-outer so one weight load feeds both
                chunks' matmuls."""
                chunks = (2 * chp, 2 * chp + 1)
                gs = [NCH * b + ch for ch in chunks]
                if chp == 0:
                    qz[b] = [sp.tile([128, T], bf16, tag=f"qz{h}", name=f"qz{h}_{b}", bufs=2)
                             for h in range(HPC)]
                    kt[b] = sp.tile([128, T], bf16, tag="kt", name=f"kt{b}", bufs=2)
                    # zero the dead halves (read by the K=128 scores matmuls).
                    # Only the first two batches: the tag rotates over 2 slots
                    # and the zero halves are never overwritten, so later
                    # batches inherit them.
                    if b < 2:
                        nc.vector.memset(qz[b][0][DH:128, :], 0.0)
                        nc.vector.memset(qz[b][1][0:DH, :], 0.0)
                # all 8 k-tiles of the chunk pair in one super-tile
                xt_s = sp.tile([128, KT * 2 * QC], bf16, tag="xt", name=f"xt{b}_{chp}", bufs=2)
                xts = [xt_s[:, k * 2 * QC:(k + 1) * 2 * QC] for k in range(KT)]
                src = xT_d[:].rearrange("(k p) t -> p k t", k=KT)[:, :, gs[0] * QC:(gs[0] + 2) * QC]
                x3 = xt_s[:].rearrange("p (k t) -> p k t", k=KT)

                def load_x():
                    if b == 0:
                        # startup: spread the k-tiles over parallel queues so
                        # the first projection matmuls start ASAP (pair 1 too:
                        # it's consumed within ~30us, before a 2-trigger load
                        # could finish). The first k-tiles of pair 0 go in
                        # half-chunk pieces so the very first matmul's input
                        # lands in ~1us of transfer.
                        if chp == 0:
                            for k in range(4):
                                eng = (nc.sync, nc.scalar, nc.gpsimd)[k % 3]
                                eng.dma_start(out=x3[:, k, 0:QC], in_=src[:, k, 0:QC])
                                eng.dma_start(out=x3[:, k, QC:2 * QC], in_=src[:, k, QC:2 * QC])
                            for k in range(4, KT):
                                eng = (nc.sync, nc.scalar, nc.gpsimd)[k % 3]
                                eng.dma_start(out=x3[:, k, :], in_=src[:, k, :])
                        else:
                            for k in range(KT):
                                eng = (nc.sync, nc.scalar, nc.gpsimd)[k % 3]
                                eng.dma_start(out=x3[:, k, :], in_=src[:, k, :])
                    else:
                        nc.sync.dma_start(out=x3[:, 0:KT // 2, :], in_=src[:, 0:KT // 2, :])
                        nc.gpsimd.dma_start(out=x3[:, KT // 2:KT, :], in_=src[:, KT // 2:KT, :])
                steps = [load_x]

                # Q/K: k-outer over the chunk pair; two PSUM accumulators
                def evac_q(pps, ci, ch):
                    for h in range(HPC):
                        nc.vector.tensor_copy(qz[b][h][h * DH:(h + 1) * DH, ch * QC:(ch + 1) * QC],
                                              pps[ci][h * DH:(h + 1) * DH, :])

                def evac_k(pps, ci, ch):
                    nc.vector.tensor_copy(kt[b][:, ch * QC:(ch + 1) * QC], pps[ci][:])

                for wt, evac, nm in ((WQT_t, evac_q, "q"), (WKT_t, evac_k, "k")):
                    pps = [ps.tile([128, QC], f32, tag="mm", name=f"pp{nm}{g}", bufs=2) for g in gs]
                    for k0 in range(0, KT, 2):
                        def fqk(wt=wt, evac=evac, k0=k0, pps=pps):
                            for k in (k0, k0 + 1):
                                for ci in range(2):
                                    nc.tensor.matmul(pps[ci][:], wt[:, k * CS:(k + 1) * CS],
                                                     xts[k][:, ci * QC:(ci + 1) * QC],
                                                     start=(k == 0), stop=(k == KT - 1))
                            if k0 + 2 == KT:
                                for ci, ch in enumerate(chunks):
                                    evac(pps, ci, ch)
                        steps.append(fqk)

                # V' directly token-major: x-block stationary, W_V.T moving.
                # out [128 tokens, 128 dims] per token-block, 4 blocks per
                # chunk packed into one [128, 512] PSUM tile as column stripes.
                for ci, ch in enumerate(chunks):
                    g = gs[ci]
                    vps = ps.tile([128, QC], f32, tag="mm", name=f"vps{g}", bufs=2)
                    vpt = sp.tile([128, 4 * 2 * 128], bf16, tag="vp", name=f"vp{b}_{ch}", bufs=2 * NCH)
                    vp[(b, ch)] = vpt
                    for tt0 in range(0, 4, 2):
                        def fv(tt0=tt0, vps=vps, vpt=vpt, ci=ci, g=g):
                            for tt in (tt0, tt0 + 1):
                                for k in range(KT):
                                    nc.tensor.matmul(vps[:, tt * 128:(tt + 1) * 128],
                                                     xts[k][:, ci * QC + tt * 128:ci * QC + (tt + 1) * 128],
                                                     WVT_t[:, k * CS:(k + 1) * CS],
                                                     start=(k == 0), stop=(k == KT - 1))
                            if tt0 == 2:
                                vp4s = vps[:].rearrange("p (t h x) -> p t h x", t=4, h=2)
                                vp4 = vpt[:].rearrange("p (t h x) -> p t h x", t=4, h=2)
                                nc.vector.tensor_copy(vp4[:, :, :, 0:DH], vp4s[:, :, :, 0:DH])
                                nc.gpsimd.tensor_copy(
                                    vp4[:, :, :, DH:DH + 1],
                                    ones_t[:, 0:8].rearrange("p (t h x) -> p t h x", t=4, h=2))
                        steps.append(fv)
                return steps

            pending = []  # queued proj closures, interleaved into attention

            def pull(n):
                for _ in range(min(n, len(pending))):
                    pending.pop(0)()

            def oproj_steps(g, ao, final=False):
                """O-projection for chunk g as filler steps (2 output tiles each).
                PSUM evacuation alternates Vector/Scalar to split the load.
                Output DMA is consolidated to 2 triggers per chunk (one per
                half), except the final chunk which streams per-tile so the
                kernel tail stays short."""
                ot = sp.tile([128, 8 * QC], bf16, tag="ot", name=f"ot{g}", bufs=2)
                dst = out_d[:].rearrange("(m p) t -> p m t", m=8)[:, :, g * QC:(g + 1) * QC]
                steps = []
                for mt in range(8):
                    def fo(mt=mt):
                        op = ps.tile([128, QC], f32, tag="mm", name=f"op{g}_{mt}", bufs=2)
                        nc.tensor.matmul(op[:], WOT_t[:, mt * 128:(mt + 1) * 128], ao[:],
                                         start=True, stop=True)
                        osl = ot[:, mt * QC:(mt + 1) * QC]
                        if mt % 2 == 1:
                            nc.scalar.copy(osl, op[:])
                        else:
                            nc.vector.tensor_copy(osl, op[:])
                        if final:
                            nc.sync.dma_start(out=dst[:, mt, :], in_=osl)
                        elif mt == 3:
                            nc.sync.dma_start(
                                out=dst[:, 0:4, :],
                                in_=ot[:, 0:4 * QC].rearrange("p (m t) -> p m t", m=4))
                        elif mt == 7:
                            nc.gpsimd.dma_start(
                                out=dst[:, 4:8, :],
                                in_=ot[:, 4 * QC:].rearrange("p (m t) -> p m t", m=4))
                    steps.append(fo)
                return steps

            def attn_chunk(b, ch, oproj_prev, pos_left=40):
                """Attention + normalize for query chunk ch of batch b.
                Two-stage software pipeline: scores/exp for kb+1 are issued before
                the PV matmuls of kb, so the PV weight-loads never wait on exp.
                oproj_prev = (g, ao) of the previous chunk, interleaved here."""
                g = NCH * b + ch
                pvs = [ps.tile([128, QC], f32, tag=f"pv{h}", name=f"pv{h}_{g}", bufs=1) for h in range(HPC)]
                nkb = 4 * ch + 4

                def scores(kb):
                    off = max(0, 128 * kb - QC * ch)
                    sc = ps.tile([128, 2 * QC], f32, tag="sc", name=f"sc{g}_{kb}", bufs=2)
                    pt = sp.tile([128, 2 * QC], bf16, tag="pt", name=f"pt{g}_{kb}", bufs=6)
                    for h in range(HPC):
                        hb = h * QC
                        # full K=128 contraction: head h's dead 64 rows of qz are
                        # zero, so the shared K stationary contributes nothing
                        # from the other head's rows
                        nc.tensor.matmul(sc[:, hb + off:hb + QC],
                                         kt[b][:, kb * 128:(kb + 1) * 128],
                                         qz[b][h][:, ch * QC + off:(ch + 1) * QC],
                                         start=True, stop=True)
                    if off == 0:
                        nc.scalar.activation(pt[:], sc[:], AFT.Exp, scale=SCALE)
                    else:
                        sc3 = sc[:].rearrange("p (h x) -> p h x", h=2)[:, :, off:QC]
                        pt3e = pt[:].rearrange("p (h x) -> p h x", h=2)[:, :, off:QC]
                        nc.scalar.activation(pt3e, sc3, AFT.Exp, scale=SCALE)
                    if 128 * kb >= QC * ch:  # diagonal block: mask keys > queries
                        pt3 = pt[:].rearrange("p (h x) -> p h x", h=2)[:, :, off:off + 128]
                        nc.vector.tensor_mul(pt3, pt3, umask_t[:].rearrange("p (o x) -> p o x", o=1).broadcast_to([128, 2, 128]))
                    return pt, off

                def pv_mm(kb, pt, off):
                    vch, vtt = kb // 4, kb % 4
                    vpt = vp[(b, vch)][:].rearrange("p (t h x) -> p t h x", t=4, h=2)
                    for h in range(HPC):
                        hb = h * QC
                        nc.tensor.matmul(pvs[h][:, off:QC],
                                         vpt[:, vtt, h, :],
                                         pt[:, hb + off:hb + QC],
                                         start=(kb == 0), stop=(kb == nkb - 1))

                if oproj_prev is not None:
                    # back of the queue: O-proj has no same-chunk consumers, so it
                    # backlogs into the last batch's attention (which has no next
                    # projection to interleave and is otherwise exp-bound)
                    pending.extend(oproj_steps(*oproj_prev))
                q0 = scores(0)
                q1 = scores(1)
                for kb in range(2, nkb):
                    cur = scores(kb)
                    # pace the projection/O-proj filler so the PE has dense
                    # work through the exp-bound deep chunks of the batch
                    pull(2 if len(pending) >= pos_left - (kb - 2) else 1)
                    pv_mm(kb - 2, *q0)
                    q0, q1 = q1, cur
                pull(1)
                pv_mm(nkb - 2, *q0)
                pull(1)
                pv_mm(nkb - 1, *q1)
                # normalize -> attnout [128, 512] bf16; one reciprocal per chunk
                ao = sp.tile([128, QC], bf16, tag="ao", name=f"ao{g}", bufs=8)
                s_g = sp.tile([1, 2 * QC], f32, tag="sh", name=f"sh{g}", bufs=3)
                r_g = sp.tile([1, 2 * QC], f32, tag="rh", name=f"rh{g}", bufs=3)
                for h in range(HPC):
                    nc.vector.tensor_copy(s_g[0:1, h * QC:(h + 1) * QC], pvs[h][64:65, :])
                nc.vector.reciprocal_approx_fast(out=r_g[0:1, :], in_=s_g[0:1, :])
                for h in range(HPC):
                    bc = sp.tile([DH, QC], f32, tag="bc", name=f"bc{g}_{h}", bufs=3)
                    nc.gpsimd.partition_broadcast(bc[:], r_g[0:1, h * QC:(h + 1) * QC])
                    nc.vector.tensor_mul(ao[h * DH:(h + 1) * DH, :], pvs[h][0:DH, :], bc[:])
                return (g, ao)

            # emission: proj(0) pair 0 eagerly, then attention starts right away
            # (chunk 0 only needs pair-0 results); pair 1 interleaves into the
            # early attention chunks via the pending queue. Both pairs' x loads
            # are issued first so pair 1's DMA overlaps pair 0's matmuls.
            p0 = proj_steps(0, 0)
            p1 = proj_steps(0, 1)
            p0[0]()
            p1[0]()
            for s in p0[1:]:
                s()
            pending.extend(p1[1:])
            oprev = None
            for b in range(B):
                if b + 1 < B:
                    for chp in range(NCH // 2):
                        pending.extend(proj_steps(b + 1, chp))
                for ch in range(NCH):
                    pos_left = sum(4 * c + 4 for c in range(ch, NCH))
                    oprev = attn_chunk(b, ch, oprev, pos_left)
                # drain most pending work, but hold tail steps back as PE filler
                # across the batch boundary (late-chunk proj of b+1 and O-proj
                # backlog, none needed by b+1's first attention chunk)
                pull(max(0, len(pending) - 16))
            pull(len(pending))  # leftover O-proj backlog
            for s in oproj_steps(*oprev, final=True):
                s()

    nc.compile()
    _cache["nc"] = nc
    return nc


def kernel(x, W_Q, W_K, W_V, W_O):
    nc = _build()
    bf = ml_dtypes.bfloat16
    xT = np.ascontiguousarray(np.asarray(x, dtype=np.float32).reshape(TOK, D).T).astype(bf)
    umask = np.triu(np.ones((128, 128), dtype=np.float32)).astype(bf)
    in_maps = []
    for c in range(NC):
        cs = slice(c * CS, (c + 1) * CS)
        in_maps.append({
            "xT": xT,
            "WQT": np.ascontiguousarray(np.asarray(W_Q, dtype=np.float32)[cs].T).astype(bf),
            "WKT": np.ascontiguousarray(np.asarray(W_K, dtype=np.float32)[cs].T).astype(bf),
            "WVT": np.ascontiguousarray(np.asarray(W_V, dtype=np.float32)[cs].T).astype(bf),
            "WOT": np.ascontiguousarray(np.asarray(W_O, dtype=np.float32)[:, cs].T).astype(bf),
            "umask": umask,
            "onesc": np.ones((128, 8), dtype=np.float32).astype(bf),
        })
    trace = bool(os.environ.get("KERNEL_TRACE"))
    res = bass_utils.run_bass_kernel_spmd(nc, in_maps, list(range(NC)), trace=trace)
    kernel.last_result = res
    out = np.zeros((D, TOK), dtype=np.float64)
    for c in range(NC):
        out += res.results[c]["outT"].astype(np.float64)
    return np.ascontiguousarray(out.T.reshape(B, T, D)).astype(np.float32)


# revision 31
# speedup vs baseline: 1.0688x; 1.0279x over previous
"""Causal self-attention (B=4, T=2048, D=1024, 16 heads) on 8 TRN2 NeuronCores.

Sharding: tensor-parallel over heads — each core owns 2 heads (a 128-dim slice
of the QKV projections, column-parallel) and the matching 128 rows of W_O
(row-parallel). Each core computes a full-shape partial output; the host sums
the 8 partials.

Per-core dataflow (bf16 operands everywhere; fp32 PSUM accumulation):
  x.T [1024, 8192] bf16 (host-converted, streamed in 512-token chunks)
  Q.T/K.T = W.T-slice @ x.T  -> [128, 2048] per batch (d-major), computed
            k-tile-outer over chunk pairs so each weight load feeds 2 matmuls
  V'       = x-chunk-block (stationary) @ W_V.T-tile (moving) -> [128 tokens,
            128 dims] directly token-major: no PE transpose needed. Stored as
            a [128, 4*2*128] bf16 super-tile per chunk: per (key-block, head)
            a 128-col padded stationary slice (cols 0-63 V, col 64 ones for
            softmax sums, 65-127 junk -> junk PV rows never read).
  S.T     = K-block @ Q.T-chunk -> [128 keys, <=512 queries] per block, bf16
            at exact causal width
  P.T     = exp(S.T / 8) -> bf16 (ScalarE; no max-subtraction: scores ~
            N(0,1)); causal: above-diagonal blocks skipped, diagonal blocks
            masked by a constant upper-triangular 0/1 bf16 multiply (2x DVE)
  out.T   = V'.T @ P.T (bf16) accumulated over key blocks -> [128, 512] PSUM
            (row 64 = softmax sums, rows 65-127 junk)
  normalize: one reciprocal per chunk -> partition-broadcast per head ->
            multiply into bf16 ao
  out_partial.T = W_O-slice.T-chunk @ attnout -> [1024, 8192] bf16 streamed
            out (PSUM->SBUF evacuation alternates Vector/Scalar engines);
            host upcasts and sums the 8 partials

Projection matmuls for batch b+1 are interleaved between attention positions
of batch b to keep the tensor engine HAM-warm at 2.4 GHz.
"""
import os
import numpy as np
import ml_dtypes
import concourse.bacc as bacc
import concourse.mybir as mybir
import concourse.tile as tile
from concourse import bass_utils

B, T, D = 4, 2048, 1024
NH, DH = 16, 64
NC = 8
HPC = NH // NC        # 2 heads per core
CS = HPC * DH         # 128 projection dims per core
TOK = B * T           # 8192 tokens
QC = 512              # query-chunk width
NCH = T // QC         # 4 chunks per batch
KT = D // 128         # 8 contraction tiles
NKB = T // 128        # 16 key blocks per batch
f32 = mybir.dt.float32
bf16 = mybir.dt.bfloat16
AFT = mybir.ActivationFunctionType
SCALE = float(1.0 / np.sqrt(DH))

_cache = {}


def _build():
    if "nc" in _cache:
        return _cache["nc"]
    nc = bacc.Bacc("TRN2", target_bir_lowering=False, debug=False)

    xT_d = nc.dram_tensor("xT", [D, TOK], bf16, kind="ExternalInput").ap()
    WQT_d = nc.dram_tensor("WQT", [D, CS], bf16, kind="ExternalInput").ap()
    WKT_d = nc.dram_tensor("WKT", [D, CS], bf16, kind="ExternalInput").ap()
    WVT_d = nc.dram_tensor("WVT", [D, CS], bf16, kind="ExternalInput").ap()
    WOT_d = nc.dram_tensor("WOT", [CS, D], bf16, kind="ExternalInput").ap()
    umask_d = nc.dram_tensor("umask", [128, 128], bf16, kind="ExternalInput").ap()
    ones_d = nc.dram_tensor("onesc", [128, 8], bf16, kind="ExternalInput").ap()
    out_d = nc.dram_tensor("outT", [D, TOK], bf16, kind="ExternalOutput").ap()

    with tile.TileContext(nc) as tc:
      with nc.allow_low_precision(reason="bf16 attention"):
        with tc.tile_pool(name="sb", bufs=1) as sb, \
             tc.tile_pool(name="sp", bufs=2) as sp, \
             tc.tile_pool(name="ps", bufs=1, space="PSUM") as ps:
            # ---- constants / weights (persistent)
            WQT_t = sb.tile([128, KT * CS], bf16, tag="wqt")
            WKT_t = sb.tile([128, KT * CS], bf16, tag="wkt")
            WVT_t = sb.tile([128, KT * CS], bf16, tag="wvt")
            # one trigger per weight tensor (3D access pattern folds the
            # k-tiles); off the sync queue: sync carries the latency-critical
            # first x-chunk loads at startup
            # startup choreography: each queue's first trigger is what the first
            # matmuls need (WQT k0 on gpsimd, x k0 on sync). The first Q matmul
            # needs only WQT k0/k1, so WQT stays per-k-pair granular.
            for k0 in range(0, KT, 2):
                nc.gpsimd.dma_start(out=WQT_t[:, k0 * CS:(k0 + 2) * CS].rearrange("p (k c) -> p k c", k=2),
                                    in_=WQT_d[k0 * 128:(k0 + 2) * 128, :].rearrange("(k p) c -> p k c", k=2))
            for k0 in range(0, KT, 4):
                nc.scalar.dma_start(out=WKT_t[:, k0 * CS:(k0 + 4) * CS].rearrange("p (k c) -> p k c", k=4),
                                    in_=WKT_d[k0 * 128:(k0 + 4) * 128, :].rearrange("(k p) c -> p k c", k=4))
            nc.gpsimd.dma_start(out=WVT_t[:].rearrange("p (k c) -> p k c", k=KT),
                                in_=WVT_d[:].rearrange("(k p) c -> p k c", k=KT))
            WOT_t = sb.tile([128, D], bf16, tag="wot")
            nc.gpsimd.dma_start(out=WOT_t[:], in_=WOT_d[:, :])
            umask_t = sb.tile([128, 128], bf16, tag="umask")
            nc.scalar.dma_start(out=umask_t[:], in_=umask_d[:, :])
            ones_t = sb.tile([128, 8], bf16, tag="ones")
            nc.scalar.dma_start(out=ones_t[:], in_=ones_d[:, :])
            # warm the ScalarE exp table set during the startup DMA wait so the
            # first real exp doesn't pay the ~2.7us ACT_TABLE_LOAD
            warm = sp.tile([1, 2], f32, tag="warm", bufs=1)
            nc.scalar.activation(warm[:], ones_t[0:1, 0:2], AFT.Exp, scale=1.0)

            qz = {}   # per-batch zero-padded Q.T pair: qz[b][h] [128, T] bf16
                      # (head h's 64 rows live, other 64 rows zero, so scores
                      # run as full K=128 matmuls sharing one K stationary)
            kt = {}   # per-batch K.T [128, T]
            vp = {}   # (b, ch) -> V' super-tile [128, 4*2*128] bf16

            def proj_steps(b, chp):
                """QKV projection for chunk pair chp (chunks 2chp, 2chp+1) of
                batch b as emit-closures, interleaved between attention
                positions. Q/K run k-tile-outer so one weight load feeds both
                chunks' matmuls."""
                chunks = (2 * chp, 2 * chp + 1)
                gs = [NCH * b + ch for ch in chunks]
                if chp == 0:
                    qz[b] = [sp.tile([128, T], bf16, tag=f"qz{h}", name=f"qz{h}_{b}", bufs=2)
                             for h in range(HPC)]
                    kt[b] = sp.tile([128, T], bf16, tag="kt", name=f"kt{b}", bufs=2)
                    # zero the dead halves (read by the K=128 scores matmuls).
                    # Only the first two batches: the tag rotates over 2 slots
                    # and the zero halves are never overwritten, so later
                    # batches inherit them.
                    if b < 2:
                        nc.vector.memset(qz[b][0][DH:128, :], 0.0)
                        nc.vector.memset(qz[b][1][0:DH, :], 0.0)
                # all 8 k-tiles of the chunk pair in one super-tile
                xt_s = sp.tile([128, KT * 2 * QC], bf16, tag="xt", name=f"xt{b}_{chp}", bufs=2)
                xts = [xt_s[:, k * 2 * QC:(k + 1) * 2 * QC] for k in range(KT)]
                src = xT_d[:].rearrange("(k p) t -> p k t", k=KT)[:, :, gs[0] * QC:(gs[0] + 2) * QC]
                x3 = xt_s[:].rearrange("p (k t) -> p k t", k=KT)

                def load_x():
                    if b == 0:
                        # startup: spread the k-tiles over parallel queues so
                        # the first projection matmuls start ASAP (pair 1 too:
                        # it's consumed within ~30us, before a 2-trigger load
                        # could finish). The first k-tiles of pair 0 go in
                        # half-chunk pieces so the very first matmul's input
                        # lands in ~1us of transfer.
                        if chp == 0:
                            for k in range(4):
                                eng = (nc.sync, nc.scalar, nc.gpsimd)[k % 3]
                                eng.dma_start(out=x3[:, k, 0:QC], in_=src[:, k, 0:QC])
                                eng.dma_start(out=x3[:, k, QC:2 * QC], in_=src[:, k, QC:2 * QC])
                            for k in range(4, KT):
                                eng = (nc.sync, nc.scalar, nc.gpsimd)[k % 3]
                                eng.dma_start(out=x3[:, k, :], in_=src[:, k, :])
                        else:
                            for k in range(KT):
                                eng = (nc.sync, nc.scalar, nc.gpsimd)[k % 3]
                                eng.dma_start(out=x3[:, k, :], in_=src[:, k, :])
                    else:
                        nc.sync.dma_start(out=x3[:, 0:KT // 2, :], in_=src[:, 0:KT // 2, :])
                        nc.gpsimd.dma_start(out=x3[:, KT // 2:KT, :], in_=src[:, KT // 2:KT, :])
                steps = [load_x]

                # Q/K: k-outer over the chunk pair; two PSUM accumulators
                def evac_q(pps, ci, ch):
                    for h in range(HPC):
                        nc.vector.tensor_copy(qz[b][h][h * DH:(h + 1) * DH, ch * QC:(ch + 1) * QC],
                                              pps[ci][h * DH:(h + 1) * DH, :])

                def evac_k(pps, ci, ch):
                    nc.vector.tensor_copy(kt[b][:, ch * QC:(ch + 1) * QC], pps[ci][:])

                for wt, evac, nm in ((WQT_t, evac_q, "q"), (WKT_t, evac_k, "k")):
                    pps = [ps.tile([128, QC], f32, tag="mm", name=f"pp{nm}{g}", bufs=2) for g in gs]
                    for k0 in range(0, KT, 2):
                        def fqk(wt=wt, evac=evac, k0=k0, pps=pps):
                            for k in (k0, k0 + 1):
                                for ci in range(2):
                                    nc.tensor.matmul(pps[ci][:], wt[:, k * CS:(k + 1) * CS],
                                                     xts[k][:, ci * QC:(ci + 1) * QC],
                                                     start=(k == 0), stop=(k == KT - 1))
                            if k0 + 2 == KT:
                                for ci, ch in enumerate(chunks):
                                    evac(pps, ci, ch)
                        steps.append(fqk)

                # V' directly token-major: x-block stationary, W_V.T moving.
                # out [128 tokens, 128 dims] per token-block, 4 blocks per
                # chunk packed into one [128, 512] PSUM tile as column stripes.
                for ci, ch in enumerate(chunks):
                    g = gs[ci]
                    vps = ps.tile([128, QC], f32, tag="mm", name=f"vps{g}", bufs=2)
                    vpt = sp.tile([128, 4 * 2 * 128], bf16, tag="vp", name=f"vp{b}_{ch}", bufs=2 * NCH)
                    vp[(b, ch)] = vpt
                    for tt0 in range(0, 4, 2):
                        def fv(tt0=tt0, vps=vps, vpt=vpt, ci=ci, g=g):
                            for tt in (tt0, tt0 + 1):
                                for k in range(KT):
                                    nc.tensor.matmul(vps[:, tt * 128:(tt + 1) * 128],
                                                     xts[k][:, ci * QC + tt * 128:ci * QC + (tt + 1) * 128],
                                                     WVT_t[:, k * CS:(k + 1) * CS],
                                                     start=(k == 0), stop=(k == KT - 1))
                            if tt0 == 2:
                                vp4s = vps[:].rearrange("p (t h x) -> p t h x", t=4, h=2)
                                vp4 = vpt[:].rearrange("p (t h x) -> p t h x", t=4, h=2)
                                nc.vector.tensor_copy(vp4[:, :, :, 0:DH], vp4s[:, :, :, 0:DH])
                                nc.gpsimd.tensor_copy(
                                    vp4[:, :, :, DH:DH + 1],
                                    ones_t[:, 0:8].rearrange("p (t h x) -> p t h x", t=4, h=2))
                        steps.append(fv)
                return steps

            pending = []  # queued proj closures, interleaved into attention

            def pull(n):
                for _ in range(min(n, len(pending))):
                    pending.pop(0)()

            def oproj_steps(g, ao, final=False):
                """O-projection for chunk g as filler steps (2 output tiles each).
                PSUM evacuation alternates Vector/Scalar to split the load.
                Output DMA is consolidated to 2 triggers per chunk (one per
                half), except the final chunk which streams per-tile so the
                kernel tail stays short."""
                ot = sp.tile([128, 8 * QC], bf16, tag="ot", name=f"ot{g}", bufs=2)
                dst = out_d[:].rearrange("(m p) t -> p m t", m=8)[:, :, g * QC:(g + 1) * QC]
                steps = []
                for mt in range(8):
                    def fo(mt=mt):
                        op = ps.tile([128, QC], f32, tag="mm", name=f"op{g}_{mt}", bufs=2)
                        nc.tensor.matmul(op[:], WOT_t[:, mt * 128:(mt + 1) * 128], ao[:],
                                         start=True, stop=True)
                        osl = ot[:, mt * QC:(mt + 1) * QC]
                        # evacuation stays off the ScalarE queue: exp is the
                        # attention block-rate limiter and ACT is strict FIFO
                        nc.vector.tensor_copy(osl, op[:])
                        if final:
                            nc.sync.dma_start(out=dst[:, mt, :], in_=osl)
                        elif mt == 3:
                            nc.sync.dma_start(
                                out=dst[:, 0:4, :],
                                in_=ot[:, 0:4 * QC].rearrange("p (m t) -> p m t", m=4))
                        elif mt == 7:
                            nc.gpsimd.dma_start(
                                out=dst[:, 4:8, :],
                                in_=ot[:, 4 * QC:].rearrange("p (m t) -> p m t", m=4))
                    steps.append(fo)
                return steps

            def attn_chunk(b, ch, oproj_prev, pos_left=40):
                """Attention + normalize for query chunk ch of batch b.
                Two-stage software pipeline: scores/exp for kb+1 are issued before
                the PV matmuls of kb, so the PV weight-loads never wait on exp.
                oproj_prev = (g, ao) of the previous chunk, interleaved here."""
                g = NCH * b + ch
                pvs = [ps.tile([128, QC], f32, tag=f"pv{h}", name=f"pv{h}_{g}", bufs=1) for h in range(HPC)]
                nkb = 4 * ch + 4

                def scores(kb):
                    off = max(0, 128 * kb - QC * ch)
                    sc = ps.tile([128, 2 * QC], f32, tag="sc", name=f"sc{g}_{kb}", bufs=2)
                    pt = sp.tile([128, 2 * QC], bf16, tag="pt", name=f"pt{g}_{kb}", bufs=6)
                    for h in range(HPC):
                        hb = h * QC
                        # full K=128 contraction: head h's dead 64 rows of qz are
                        # zero, so the shared K stationary contributes nothing
                        # from the other head's rows
                        nc.tensor.matmul(sc[:, hb + off:hb + QC],
                                         kt[b][:, kb * 128:(kb + 1) * 128],
                                         qz[b][h][:, ch * QC + off:(ch + 1) * QC],
                                         start=True, stop=True)
                    if off == 0:
                        nc.scalar.activation(pt[:], sc[:], AFT.Exp, scale=SCALE)
                    else:
                        sc3 = sc[:].rearrange("p (h x) -> p h x", h=2)[:, :, off:QC]
                        pt3e = pt[:].rearrange("p (h x) -> p h x", h=2)[:, :, off:QC]
                        nc.scalar.activation(pt3e, sc3, AFT.Exp, scale=SCALE)
                    if 128 * kb >= QC * ch:  # diagonal block: mask keys > queries
                        pt3 = pt[:].rearrange("p (h x) -> p h x", h=2)[:, :, off:off + 128]
                        nc.vector.tensor_mul(pt3, pt3, umask_t[:].rearrange("p (o x) -> p o x", o=1).broadcast_to([128, 2, 128]))
                    return pt, off

                def pv_mm(kb, pt, off):
                    vch, vtt = kb // 4, kb % 4
                    vpt = vp[(b, vch)][:].rearrange("p (t h x) -> p t h x", t=4, h=2)
                    for h in range(HPC):
                        hb = h * QC
                        nc.tensor.matmul(pvs[h][:, off:QC],
                                         vpt[:, vtt, h, :],
                                         pt[:, hb + off:hb + QC],
                                         start=(kb == 0), stop=(kb == nkb - 1))

                if oproj_prev is not None:
                    # back of the queue: O-proj has no same-chunk consumers, so it
                    # backlogs into the last batch's attention (which has no next
                    # projection to interleave and is otherwise exp-bound)
                    pending.extend(oproj_steps(*oproj_prev))
                q0 = scores(0)
                q1 = scores(1)
                for kb in range(2, nkb):
                    cur = scores(kb)
                    pull(2)
                    pv_mm(kb - 2, *q0)
                    q0, q1 = q1, cur
                pull(1)
                pv_mm(nkb - 2, *q0)
                pull(1)
                pv_mm(nkb - 1, *q1)
                # normalize -> attnout [128, 512] bf16; one reciprocal per chunk
                ao = sp.tile([128, QC], bf16, tag="ao", name=f"ao{g}", bufs=8)
                s_g = sp.tile([1, 2 * QC], f32, tag="sh", name=f"sh{g}", bufs=3)
                r_g = sp.tile([1, 2 * QC], f32, tag="rh", name=f"rh{g}", bufs=3)
                for h in range(HPC):
                    nc.vector.tensor_copy(s_g[0:1, h * QC:(h + 1) * QC], pvs[h][64:65, :])
                nc.vector.reciprocal_approx_fast(out=r_g[0:1, :], in_=s_g[0:1, :])
                for h in range(HPC):
                    bc = sp.tile([DH, QC], f32, tag="bc", name=f"bc{g}_{h}", bufs=3)
                    nc.gpsimd.partition_broadcast(bc[:], r_g[0:1, h * QC:(h + 1) * QC])
                    nc.vector.tensor_mul(ao[h * DH:(h + 1) * DH, :], pvs[h][0:DH, :], bc[:])
                return (g, ao)

            # emission: proj(0) pair 0 eagerly, then attention starts right away
            # (chunk 0 only needs pair-0 results); pair 1 interleaves into the
            # early attention chunks via the pending queue. Both pairs' x loads
            # are issued first so pair 1's DMA overlaps pair 0's matmuls.
            p0 = proj_steps(0, 0)
            p1 = proj_steps(0, 1)
            p0[0]()
            p1[0]()
            for s in p0[1:]:
                s()
            pending.extend(p1[1:])
            oprev = None
            for b in range(B):
                if b + 1 < B:
                    for chp in range(NCH // 2):
                        pending.extend(proj_steps(b + 1, chp))
                for ch in range(NCH):
                    pos_left = sum(4 * c + 4 for c in range(ch, NCH))
                    oprev = attn_chunk(b, ch, oprev, pos_left)
                # drain most pending work, but hold tail steps back as PE filler
                # across the batch boundary (late-chunk proj of b+1 and O-proj
                # backlog, none needed by b+1's first attention chunk)
                pull(max(0, len(pending) - 16))
            pull(len(pending))  # leftover O-proj backlog
            for s in oproj_steps(*oprev, final=True):
                s()

    nc.compile()
    _cache["nc"] = nc
    return nc


def kernel(x, W_Q, W_K, W_V, W_O):
    nc = _build()
    bf = ml_dtypes.bfloat16
    xT = np.ascontiguousarray(np.asarray(x, dtype=np.float32).reshape(TOK, D).T).astype(bf)
    umask = np.triu(np.ones((128, 128), dtype=np.float32)).astype(bf)
    in_maps = []
    for c in range(NC):
        cs = slice(c * CS, (c + 1) * CS)
        in_maps.append({
            "xT": xT,
            "WQT": np.ascontiguousarray(np.asarray(W_Q, dtype=np.float32)[cs].T).astype(bf),
            "WKT": np.ascontiguousarray(np.asarray(W_K, dtype=np.float32)[cs].T).astype(bf),
            "WVT": np.ascontiguousarray(np.asarray(W_V, dtype=np.float32)[cs].T).astype(bf),
            "WOT": np.ascontiguousarray(np.asarray(W_O, dtype=np.float32)[:, cs].T).astype(bf),
            "umask": umask,
            "onesc": np.ones((128, 8), dtype=np.float32).astype(bf),
        })
    trace = bool(os.environ.get("KERNEL_TRACE"))
    res = bass_utils.run_bass_kernel_spmd(nc, in_maps, list(range(NC)), trace=trace)
    kernel.last_result = res
    out = np.zeros((D, TOK), dtype=np.float64)
    for c in range(NC):
        out += res.results[c]["outT"].astype(np.float64)
    return np.ascontiguousarray(out.T.reshape(B, T, D)).astype(np.float32)
